# revision 14
# baseline (speedup 1.0000x reference)
"""NoisyNet dense layer (training mode) on 8 TRN2 NeuronCores.

out[b,u] = x @ W_mu + eps_out * ((x*eps_in) @ W_sigma) + bias_mu + bias_sigma*eps_out

Sharding: data-parallel over batch (4096 -> 512 rows/core), weights/biases
replicated. On-device math runs in a transposed layout ([D,B]/[U,B]) so the
contraction dim D lands on SBUF partitions; the host does the (free)
transposes, bf16 casts and the final gather.

Schedule (v2): the two HWDGE rings are dedicated — sync carries the weight
stream, scalar carries x first, then eps_in/eps_out, then half the output
tiles (the other half drain on sync after the weights finish). Startup runs
k-chunk-major over the first 4 u-tiles so the PE consumes each arriving x
chunk against 4 weight tiles (8 matmuls per 256KB chunk) instead of
starving on a single u-tile k-loop. Warm-up matmuls on a vector-memset
tile cover the DMA latency + PE p-state ramp. The last two u-tiles run in
256-wide halves so the epilogue pipelines with the final matmuls.
"""

import numpy as np
import ml_dtypes

import concourse.bacc as bacc
import concourse.mybir as mybir
import concourse.tile as tile
from concourse.bass_utils import run_bass_kernel_spmd

N_CORES = 8
B, D, U = 4096, 2048, 2048
BL = B // N_CORES          # 512 batch rows per core
P = 128                    # partitions
KT = D // P                # 16 contraction tiles
UT = U // P                # 16 output tiles
XC = 8                     # x DMA chunks (2 k-tiles each)
WSL = 4                    # wm0-3 arrive in 4 k-slices each
NST = 4                    # u-tiles processed k-chunk-major at startup
BF16 = mybir.dt.bfloat16
FP32 = mybir.dt.float32

_NBF = ml_dtypes.bfloat16

_cached = None


def _build():
    nc = bacc.Bacc("TRN2", target_bir_lowering=False, debug=False)

    # activations laid out [P, KT, BL]: partition p holds d = k*128+p
    xT = nc.declare_dram_parameter("xT", [P, KT, BL], BF16, isOutput=False)
    eiT = nc.declare_dram_parameter("eiT", [P, KT, BL], BF16, isOutput=False)
    eoT = nc.declare_dram_parameter("eoT", [P, UT, BL], BF16, isOutput=False)
    wmu = nc.declare_dram_parameter("wmu", [UT, P, KT * P], BF16, isOutput=False)
    wsg = nc.declare_dram_parameter("wsg", [UT, P, KT * P], BF16, isOutput=False)
    bmu = nc.declare_dram_parameter("bmu", [P, UT], FP32, isOutput=False)
    bsg = nc.declare_dram_parameter("bsg", [P, UT], FP32, isOutput=False)
    outT = nc.declare_dram_parameter("outT", [UT, P, BL], FP32, isOutput=True)

    with tile.TileContext(nc) as tc:
        with (
            tc.tile_pool(name="acts", bufs=1) as acts,
            tc.tile_pool(name="w", bufs=6) as wp,
            tc.tile_pool(name="bias", bufs=1) as bp,
            tc.tile_pool(name="psum", bufs=1, space="PSUM") as pp,
            tc.tile_pool(name="psumn", bufs=3, space="PSUM") as ppn,
            tc.tile_pool(name="mean", bufs=1) as mp,
            tc.tile_pool(name="tmp", bufs=2) as tp,
            tc.tile_pool(name="out", bufs=3) as op,
        ):
            # ---- DMA issue (program order == ring FIFO order per engine) ----
            # sync ring: x chunks interleaved with wm0-3 k-slices, in the
            # exact order phase 1a consumes them. The sync ring's first
            # trigger fires ~1.3us before the scalar ring's (the scalar
            # engine runs ACT_TABLE_LOAD first), so the critical stream
            # lives here. Then wm4..7; wm8..15 + ws0..15 from the loops.
            x_sb = acts.tile([P, KT, BL], BF16, tag="x")
            ei_sb = acts.tile([P, KT, BL], BF16, tag="ei")
            z_sb = acts.tile([P, KT, BL], BF16, tag="z")
            eo_sb = acts.tile([P, UT, BL], BF16, tag="eo")

            wm_tiles = {}
            ws_tiles = {}
            for u in range(NST):
                wm_tiles[u] = wp.tile([P, KT * P], BF16, tag="wm", bufs=10,
                                      name=f"wm_st{u}")
            KC = KT // XC             # 2 k-tiles per x chunk
            SL = KT * P // WSL        # 512 cols per wm slice (4 k-tiles)

            def x_chunk(c):
                s = slice(c * KC, (c + 1) * KC)
                nc.sync.dma_start(x_sb[:, s, :], xT[:, s, :])

            def wm_slice(u, s):
                nc.sync.dma_start(wm_tiles[u][:, s * SL:(s + 1) * SL],
                                  wmu[u][:, s * SL:(s + 1) * SL])

            x_chunk(0)
            wm_slice(0, 0)
            wm_slice(1, 0)
            x_chunk(1)
            wm_slice(2, 0)
            wm_slice(3, 0)
            for s in range(1, WSL):
                x_chunk(2 * s)
                wm_slice(0, s)
                wm_slice(1, s)
                x_chunk(2 * s + 1)
                wm_slice(2, s)
                wm_slice(3, s)

            def fetch_wm(u):
                wm = wp.tile([P, KT * P], BF16, tag="wm", bufs=10,
                             name=f"wm{u}")
                nc.sync.dma_start(wm[:], wmu[u])
                wm_tiles[u] = wm

            def fetch_ws(u):
                ws = wp.tile([P, KT * P], BF16, tag="ws", bufs=4,
                             name=f"ws{u}")
                nc.sync.dma_start(ws[:], wsg[u])
                ws_tiles[u] = ws

            for u in range(NST, NST + 4):
                fetch_wm(u)

            def fetch_ei(c):
                s = slice(c * 4, (c + 1) * 4)
                nc.scalar.dma_start(ei_sb[:, s, :], eiT[:, s, :])
                nc.vector.tensor_mul(z_sb[:, s, :], x_sb[:, s, :], ei_sb[:, s, :])

            def fetch_eo(c):
                s = slice(c * 4, (c + 1) * 4)
                nc.scalar.dma_start(eo_sb[:, s, :], eoT[:, s, :])

            # biases (tiny) on the gpsimd SWDGE queue.
            bmu_t = bp.tile([P, UT], FP32, tag="bmu")
            nc.gpsimd.dma_start(bmu_t[:], bmu[:])
            bsg_t = bp.tile([P, UT], FP32, tag="bsg")
            nc.gpsimd.dma_start(bsg_t[:], bsg[:])

            # ---- Phase 1a: k-chunk-major over u=0..3 while x streams in ----
            t_m = [None] * UT
            pm_st = [pp.tile([P, BL], FP32, tag=f"psm{u}", name=f"pm_st{u}")
                     for u in range(NST)]
            for c in range(XC):
                for u in range(NST):
                    wm = wm_tiles[u]
                    for k in range(c * KC, (c + 1) * KC):
                        nc.tensor.matmul(
                            pm_st[u][:], wm[:, k * P:(k + 1) * P], x_sb[:, k, :],
                            start=(k == 0), stop=(k == KT - 1),
                        )
            for u in range(NST):
                del wm_tiles[u]
                tm = mp.tile([P, BL], FP32, tag=f"tm{u}")
                nc.scalar.add(tm[:], pm_st[u][:], bmu_t[:, u:u + 1])
                t_m[u] = tm

            # ---- Phase 1b: mean terms u=4..15, u-major ----
            # eps_in/eps_out stream in the gaps of the weight stream.
            ei_at = {6: 0, 8: 1, 10: 2, 12: 3}
            eo_at = {13: 0, 14: 1, 15: 2}
            for u in range(NST, UT):
                if u + 4 < UT:
                    fetch_wm(u + 4)
                elif u + 4 == UT:
                    for uu in range(2):
                        fetch_ws(uu)
                if u in ei_at:
                    fetch_ei(ei_at[u])
                if u in eo_at:
                    fetch_eo(eo_at[u])
                wm = wm_tiles.pop(u)
                # u=4..6 borrow the (still idle) noise-phase PSUM banks so
                # their k-loops don't wait on the u=0..3 ACT drains.
                if u < NST + 3:
                    pm = ppn.tile([P, BL], FP32, tag="psn")
                else:
                    pm = pp.tile([P, BL], FP32, tag=f"psm{u % NST}")
                for k in range(KT):
                    nc.tensor.matmul(
                        pm[:], wm[:, k * P:(k + 1) * P], x_sb[:, k, :],
                        start=(k == 0), stop=(k == KT - 1),
                    )
                tm = mp.tile([P, BL], FP32, tag=f"tm{u}")
                nc.scalar.add(tm[:], pm[:], bmu_t[:, u:u + 1])
                t_m[u] = tm

            # ---- Phase 2: noise terms + combine ----
            for u in range(UT):
                un = u + 2
                if 2 <= un < UT:
                    fetch_ws(un)
                if u == 0:
                    fetch_eo(3)
                ws = ws_tiles.pop(u)
                # last two tiles: batch halves so the epilogue pipelines with
                # the final matmuls instead of serializing after them.
                halves = (0, BL // 2, BL) if u >= UT - 2 else (0, BL)
                for h in range(len(halves) - 1):
                    lo, hi = halves[h], halves[h + 1]
                    last_piece = (u == UT - 1 and h == len(halves) - 2)
                    pn = ppn.tile([P, hi - lo], FP32, tag="psn")
                    for k in range(KT):
                        nc.tensor.matmul(
                            pn[:], ws[:, k * P:(k + 1) * P], z_sb[:, k, lo:hi],
                            start=(k == 0), stop=(k == KT - 1),
                        )
                    t_n = tp.tile([P, hi - lo], FP32, tag="tn")
                    nc.scalar.add(t_n[:], pn[:], bsg_t[:, u:u + 1])
                    pr = tp.tile([P, hi - lo], FP32, tag="pr")
                    nc.vector.tensor_mul(pr[:], t_n[:], eo_sb[:, u, lo:hi])
                    o = op.tile([P, hi - lo], FP32, tag="o")
                    nc.vector.tensor_add(o[:], pr[:], t_m[u][:, lo:hi])
                    # outputs drain on the scalar ring; the final piece uses
                    # the (idle) sync ring so the last two out triggers run
                    # on different sequencers in parallel.
                    if last_piece:
                        nc.sync.dma_start(outT[u][:, lo:hi], o[:])
                    else:
                        nc.scalar.dma_start(outT[u][:, lo:hi], o[:])

    nc.compile()
    return nc


def _get_nc():
    global _cached
    if _cached is None:
        _cached = _build()
    return _cached


def kernel(x, weight_mu, weight_sigma, bias_mu, bias_sigma, eps_in, eps_out,
           _trace=False):
    nc = _get_nc()

    # Host-side layout prep (transposes + bf16 casts only; no layer math).
    def to_pkb(a):  # [B, D] -> per-core [P, KT, BL] (partition p holds k*128+p)
        a = np.ascontiguousarray(a.astype(_NBF))
        return [
            np.ascontiguousarray(
                a[c * BL:(c + 1) * BL].T.reshape(KT, P, BL).transpose(1, 0, 2))
            for c in range(N_CORES)
        ]

    xs = to_pkb(x)
    eis = to_pkb(eps_in)
    eos = to_pkb(eps_out)  # same transform, u in place of k

    def w_blocks(w):  # [D, U] -> [UT, P(d within block), KT*P] bf16
        wb = w.astype(_NBF).reshape(KT, P, UT, P).transpose(2, 1, 0, 3)
        return np.ascontiguousarray(wb.reshape(UT, P, KT * P))

    wmu_h = w_blocks(weight_mu)
    wsg_h = w_blocks(weight_sigma)
    bmu_h = np.ascontiguousarray(bias_mu.astype(np.float32).reshape(UT, P).T)
    bsg_h = np.ascontiguousarray(bias_sigma.astype(np.float32).reshape(UT, P).T)

    in_maps = [
        {
            "xT": xs[c],
            "eiT": eis[c],
            "eoT": eos[c],
            "wmu": wmu_h,
            "wsg": wsg_h,
            "bmu": bmu_h,
            "bsg": bsg_h,
        }
        for c in range(N_CORES)
    ]

    res = run_bass_kernel_spmd(nc, in_maps, core_ids=list(range(N_CORES)),
                               trace=_trace)
    kernel.last_result = res

    out = np.empty((B, U), dtype=np.float32)
    for c in range(N_CORES):
        oc = res.results[c]["outT"]  # [UT, P, BL]
        out[c * BL:(c + 1) * BL] = oc.transpose(2, 0, 1).reshape(BL, U)
    return out


# revision 15
# speedup vs baseline: 1.0487x; 1.0487x over previous
"""NoisyNet dense layer (training mode) on 8 TRN2 NeuronCores.

out[b,u] = x @ W_mu + eps_out * ((x*eps_in) @ W_sigma) + bias_mu + bias_sigma*eps_out

Sharding: data-parallel over batch (4096 -> 512 rows/core), weights/biases
replicated. On-device math runs in a transposed layout ([D,B]/[U,B]) so the
contraction dim D lands on SBUF partitions; the host does the (free)
transposes, bf16 casts and the final gather.

Schedule (v2): the two HWDGE rings are dedicated — sync carries the weight
stream, scalar carries x first, then eps_in/eps_out, then half the output
tiles (the other half drain on sync after the weights finish). Startup runs
k-chunk-major over the first 4 u-tiles so the PE consumes each arriving x
chunk against 4 weight tiles (8 matmuls per 256KB chunk) instead of
starving on a single u-tile k-loop. Warm-up matmuls on a vector-memset
tile cover the DMA latency + PE p-state ramp. The last two u-tiles run in
256-wide halves so the epilogue pipelines with the final matmuls.
"""

import numpy as np
import ml_dtypes

import concourse.bacc as bacc
import concourse.mybir as mybir
import concourse.tile as tile
from concourse.bass_utils import run_bass_kernel_spmd

N_CORES = 8
B, D, U = 4096, 2048, 2048
BL = B // N_CORES          # 512 batch rows per core
P = 128                    # partitions
KT = D // P                # 16 contraction tiles
UT = U // P                # 16 output tiles
XC = 8                     # x DMA chunks (2 k-tiles each)
WSL = 4                    # wm0-3 arrive in 4 k-slices each
NST = 4                    # u-tiles processed k-chunk-major at startup
BF16 = mybir.dt.bfloat16
FP32 = mybir.dt.float32

_NBF = ml_dtypes.bfloat16

_cached = None


def _build():
    nc = bacc.Bacc("TRN2", target_bir_lowering=False, debug=False)

    # activations laid out [P, KT, BL]: partition p holds d = k*128+p
    xT = nc.declare_dram_parameter("xT", [P, KT, BL], BF16, isOutput=False)
    eiT = nc.declare_dram_parameter("eiT", [P, KT, BL], BF16, isOutput=False)
    eoT = nc.declare_dram_parameter("eoT", [P, UT, BL], BF16, isOutput=False)
    wmu = nc.declare_dram_parameter("wmu", [UT, P, KT * P], BF16, isOutput=False)
    wsg = nc.declare_dram_parameter("wsg", [UT, P, KT * P], BF16, isOutput=False)
    bmu = nc.declare_dram_parameter("bmu", [P, UT], FP32, isOutput=False)
    bsg = nc.declare_dram_parameter("bsg", [P, UT], FP32, isOutput=False)
    outT = nc.declare_dram_parameter("outT", [UT, P, BL], FP32, isOutput=True)

    with tile.TileContext(nc) as tc:
        with (
            tc.tile_pool(name="acts", bufs=1) as acts,
            tc.tile_pool(name="w", bufs=6) as wp,
            tc.tile_pool(name="bias", bufs=1) as bp,
            tc.tile_pool(name="psum", bufs=1, space="PSUM") as pp,
            tc.tile_pool(name="psumn", bufs=3, space="PSUM") as ppn,
            tc.tile_pool(name="mean", bufs=1) as mp,
            tc.tile_pool(name="tmp", bufs=2) as tp,
            tc.tile_pool(name="out", bufs=3) as op,
        ):
            # ---- DMA issue (program order == ring FIFO order per engine) ----
            # sync ring: x chunks interleaved with wm0-3 k-slices, in the
            # exact order phase 1a consumes them. The sync ring's first
            # trigger fires ~1.3us before the scalar ring's (the scalar
            # engine runs ACT_TABLE_LOAD first), so the critical stream
            # lives here. Then wm4..7; wm8..15 + ws0..15 from the loops.
            x_sb = acts.tile([P, KT, BL], BF16, tag="x")
            ei_sb = acts.tile([P, KT, BL], BF16, tag="ei")
            z_sb = acts.tile([P, KT, BL], BF16, tag="z")
            eo_sb = acts.tile([P, UT, BL], BF16, tag="eo")

            wm_tiles = {}
            ws_tiles = {}
            for u in range(NST):
                wm_tiles[u] = wp.tile([P, KT * P], BF16, tag="wm", bufs=10,
                                      name=f"wm_st{u}")
            KC = KT // XC             # 2 k-tiles per x chunk
            SL = KT * P // WSL        # 512 cols per wm slice (4 k-tiles)

            def wm_slice(u, s):
                nc.sync.dma_start(wm_tiles[u][:, s * SL:(s + 1) * SL],
                                  wmu[u][:, s * SL:(s + 1) * SL])

            # sync ring: x_c0 jumps the queue (sync's first trigger fires
            # ~1.3us before scalar's), then the wm slices slice-major.
            nc.sync.dma_start(x_sb[:, 0:KC, :], xT[:, 0:KC, :])
            for s in range(WSL):
                for u in range(NST):
                    wm_slice(u, s)
            # scalar ring: the remaining x chunks.
            for c in range(1, XC):
                s = slice(c * KC, (c + 1) * KC)
                nc.scalar.dma_start(x_sb[:, s, :], xT[:, s, :])

            def fetch_wm(u):
                wm = wp.tile([P, KT * P], BF16, tag="wm", bufs=10,
                             name=f"wm{u}")
                nc.sync.dma_start(wm[:], wmu[u])
                wm_tiles[u] = wm

            def fetch_ws(u):
                ws = wp.tile([P, KT * P], BF16, tag="ws", bufs=4,
                             name=f"ws{u}")
                nc.sync.dma_start(ws[:], wsg[u])
                ws_tiles[u] = ws

            for u in range(NST, NST + 4):
                fetch_wm(u)

            def fetch_ei(c):
                s = slice(c * 4, (c + 1) * 4)
                nc.scalar.dma_start(ei_sb[:, s, :], eiT[:, s, :])
                nc.vector.tensor_mul(z_sb[:, s, :], x_sb[:, s, :], ei_sb[:, s, :])

            def fetch_eo(c):
                s = slice(c * 4, (c + 1) * 4)
                nc.scalar.dma_start(eo_sb[:, s, :], eoT[:, s, :])

            # biases (tiny) on the gpsimd SWDGE queue.
            bmu_t = bp.tile([P, UT], FP32, tag="bmu")
            nc.gpsimd.dma_start(bmu_t[:], bmu[:])
            bsg_t = bp.tile([P, UT], FP32, tag="bsg")
            nc.gpsimd.dma_start(bsg_t[:], bsg[:])

            # ---- Phase 1a: k-chunk-major over u=0..3 while x streams in ----
            t_m = [None] * UT
            pm_st = [pp.tile([P, BL], FP32, tag=f"psm{u}", name=f"pm_st{u}")
                     for u in range(NST)]
            for c in range(XC):
                for u in range(NST):
                    wm = wm_tiles[u]
                    for k in range(c * KC, (c + 1) * KC):
                        nc.tensor.matmul(
                            pm_st[u][:], wm[:, k * P:(k + 1) * P], x_sb[:, k, :],
                            start=(k == 0), stop=(k == KT - 1),
                        )
            for u in range(NST):
                del wm_tiles[u]
                tm = mp.tile([P, BL], FP32, tag=f"tm{u}")
                nc.scalar.add(tm[:], pm_st[u][:], bmu_t[:, u:u + 1])
                t_m[u] = tm

            # ---- Phase 1b: mean terms u=4..15, u-major ----
            # eps_in/eps_out stream in the gaps of the weight stream.
            ei_at = {6: 0, 8: 1, 10: 2, 12: 3}
            eo_at = {13: 0, 14: 1, 15: 2}
            for u in range(NST, UT):
                if u + 4 < UT:
                    fetch_wm(u + 4)
                elif u + 4 == UT:
                    for uu in range(2):
                        fetch_ws(uu)
                if u in ei_at:
                    fetch_ei(ei_at[u])
                if u in eo_at:
                    fetch_eo(eo_at[u])
                wm = wm_tiles.pop(u)
                # u=4..6 borrow the (still idle) noise-phase PSUM banks so
                # their k-loops don't wait on the u=0..3 ACT drains.
                if u < NST + 3:
                    pm = ppn.tile([P, BL], FP32, tag="psn")
                else:
                    pm = pp.tile([P, BL], FP32, tag=f"psm{u % NST}")
                for k in range(KT):
                    nc.tensor.matmul(
                        pm[:], wm[:, k * P:(k + 1) * P], x_sb[:, k, :],
                        start=(k == 0), stop=(k == KT - 1),
                    )
                tm = mp.tile([P, BL], FP32, tag=f"tm{u}")
                nc.scalar.add(tm[:], pm[:], bmu_t[:, u:u + 1])
                t_m[u] = tm

            # ---- Phase 2: noise terms + combine ----
            for u in range(UT):
                un = u + 2
                if 2 <= un < UT:
                    fetch_ws(un)
                if u == 0:
                    fetch_eo(3)
                ws = ws_tiles.pop(u)
                # last two tiles: batch halves so the epilogue pipelines with
                # the final matmuls instead of serializing after them.
                halves = (0, BL // 2, BL) if u >= UT - 2 else (0, BL)
                for h in range(len(halves) - 1):
                    lo, hi = halves[h], halves[h + 1]
                    last_piece = (u == UT - 1 and h == len(halves) - 2)
                    pn = ppn.tile([P, hi - lo], FP32, tag="psn")
                    for k in range(KT):
                        nc.tensor.matmul(
                            pn[:], ws[:, k * P:(k + 1) * P], z_sb[:, k, lo:hi],
                            start=(k == 0), stop=(k == KT - 1),
                        )
                    t_n = tp.tile([P, hi - lo], FP32, tag="tn")
                    nc.scalar.add(t_n[:], pn[:], bsg_t[:, u:u + 1])
                    pr = tp.tile([P, hi - lo], FP32, tag="pr")
                    nc.vector.tensor_mul(pr[:], t_n[:], eo_sb[:, u, lo:hi])
                    o = op.tile([P, hi - lo], FP32, tag="o")
                    nc.vector.tensor_add(o[:], pr[:], t_m[u][:, lo:hi])
                    # outputs drain on the scalar ring; the final piece uses
                    # the (idle) sync ring so the last two out triggers run
                    # on different sequencers in parallel.
                    if last_piece:
                        nc.sync.dma_start(outT[u][:, lo:hi], o[:])
                    else:
                        nc.scalar.dma_start(outT[u][:, lo:hi], o[:])

    nc.compile()
    return nc


def _get_nc():
    global _cached
    if _cached is None:
        _cached = _build()
    return _cached


def kernel(x, weight_mu, weight_sigma, bias_mu, bias_sigma, eps_in, eps_out,
           _trace=False):
    nc = _get_nc()

    # Host-side layout prep (transposes + bf16 casts only; no layer math).
    def to_pkb(a):  # [B, D] -> per-core [P, KT, BL] (partition p holds k*128+p)
        a = np.ascontiguousarray(a.astype(_NBF))
        return [
            np.ascontiguousarray(
                a[c * BL:(c + 1) * BL].T.reshape(KT, P, BL).transpose(1, 0, 2))
            for c in range(N_CORES)
        ]

    xs = to_pkb(x)
    eis = to_pkb(eps_in)
    eos = to_pkb(eps_out)  # same transform, u in place of k

    def w_blocks(w):  # [D, U] -> [UT, P(d within block), KT*P] bf16
        wb = w.astype(_NBF).reshape(KT, P, UT, P).transpose(2, 1, 0, 3)
        return np.ascontiguousarray(wb.reshape(UT, P, KT * P))

    wmu_h = w_blocks(weight_mu)
    wsg_h = w_blocks(weight_sigma)
    bmu_h = np.ascontiguousarray(bias_mu.astype(np.float32).reshape(UT, P).T)
    bsg_h = np.ascontiguousarray(bias_sigma.astype(np.float32).reshape(UT, P).T)

    in_maps = [
        {
            "xT": xs[c],
            "eiT": eis[c],
            "eoT": eos[c],
            "wmu": wmu_h,
            "wsg": wsg_h,
            "bmu": bmu_h,
            "bsg": bsg_h,
        }
        for c in range(N_CORES)
    ]

    res = run_bass_kernel_spmd(nc, in_maps, core_ids=list(range(N_CORES)),
                               trace=_trace)
    kernel.last_result = res

    out = np.empty((B, U), dtype=np.float32)
    for c in range(N_CORES):
        oc = res.results[c]["outT"]  # [UT, P, BL]
        out[c * BL:(c + 1) * BL] = oc.transpose(2, 0, 1).reshape(BL, U)
    return out


# revision 16
# speedup vs baseline: 1.0637x; 1.0143x over previous
"""NoisyNet dense layer (training mode) on 8 TRN2 NeuronCores.

out[b,u] = x @ W_mu + eps_out * ((x*eps_in) @ W_sigma) + bias_mu + bias_sigma*eps_out

Sharding: data-parallel over batch (4096 -> 512 rows/core), weights/biases
replicated. On-device math runs in a transposed layout ([D,B]/[U,B]) so the
contraction dim D lands on SBUF partitions; the host does the (free)
transposes, bf16 casts and the final gather.

Schedule (v2): the two HWDGE rings are dedicated — sync carries the weight
stream, scalar carries x first, then eps_in/eps_out, then half the output
tiles (the other half drain on sync after the weights finish). Startup runs
k-chunk-major over the first 4 u-tiles so the PE consumes each arriving x
chunk against 4 weight tiles (8 matmuls per 256KB chunk) instead of
starving on a single u-tile k-loop. Warm-up matmuls on a vector-memset
tile cover the DMA latency + PE p-state ramp. The last two u-tiles run in
256-wide halves so the epilogue pipelines with the final matmuls.
"""

import numpy as np
import ml_dtypes

import concourse.bacc as bacc
import concourse.mybir as mybir
import concourse.tile as tile
from concourse.bass_utils import run_bass_kernel_spmd

N_CORES = 8
B, D, U = 4096, 2048, 2048
BL = B // N_CORES          # 512 batch rows per core
P = 128                    # partitions
KT = D // P                # 16 contraction tiles
UT = U // P                # 16 output tiles
XC = 8                     # x DMA chunks (2 k-tiles each)
WSL = 4                    # wm0-3 arrive in 4 k-slices each
NST = 4                    # u-tiles processed k-chunk-major at startup
BF16 = mybir.dt.bfloat16
FP32 = mybir.dt.float32

_NBF = ml_dtypes.bfloat16

_cached = None


def _build():
    nc = bacc.Bacc("TRN2", target_bir_lowering=False, debug=False)

    # activations laid out [P, KT, BL]: partition p holds d = k*128+p
    xT = nc.declare_dram_parameter("xT", [P, KT, BL], BF16, isOutput=False)
    eiT = nc.declare_dram_parameter("eiT", [P, KT, BL], BF16, isOutput=False)
    eoT = nc.declare_dram_parameter("eoT", [P, UT, BL], BF16, isOutput=False)
    wmu = nc.declare_dram_parameter("wmu", [UT, P, KT * P], BF16, isOutput=False)
    wsg = nc.declare_dram_parameter("wsg", [UT, P, KT * P], BF16, isOutput=False)
    bmu = nc.declare_dram_parameter("bmu", [P, UT], FP32, isOutput=False)
    bsg = nc.declare_dram_parameter("bsg", [P, UT], FP32, isOutput=False)
    outT = nc.declare_dram_parameter("outT", [UT, P, BL], FP32, isOutput=True)

    with tile.TileContext(nc) as tc:
        with (
            tc.tile_pool(name="acts", bufs=1) as acts,
            tc.tile_pool(name="w", bufs=6) as wp,
            tc.tile_pool(name="bias", bufs=1) as bp,
            tc.tile_pool(name="psum", bufs=1, space="PSUM") as pp,
            tc.tile_pool(name="psumn", bufs=3, space="PSUM") as ppn,
            tc.tile_pool(name="mean", bufs=1) as mp,
            tc.tile_pool(name="tmp", bufs=2) as tp,
            tc.tile_pool(name="out", bufs=3) as op,
        ):
            # ---- DMA issue (program order == ring FIFO order per engine) ----
            # sync ring: x chunks interleaved with wm0-3 k-slices, in the
            # exact order phase 1a consumes them. The sync ring's first
            # trigger fires ~1.3us before the scalar ring's (the scalar
            # engine runs ACT_TABLE_LOAD first), so the critical stream
            # lives here. Then wm4..7; wm8..15 + ws0..15 from the loops.
            x_sb = acts.tile([P, KT, BL], BF16, tag="x")
            ei_sb = acts.tile([P, KT, BL], BF16, tag="ei")
            z_sb = acts.tile([P, KT, BL], BF16, tag="z")
            eo_sb = acts.tile([P, UT, BL], BF16, tag="eo")

            wm_tiles = {}
            ws_tiles = {}
            for u in range(NST):
                wm_tiles[u] = wp.tile([P, KT * P], BF16, tag="wm", bufs=10,
                                      name=f"wm_st{u}")
            KC = KT // XC             # 2 k-tiles per x chunk
            SL = KT * P // 2          # 1024 cols per wm slice (8 k-tiles)

            # sync ring: wm0-3 in 2 k-slices each, slice-major so the
            # startup phase gets all 4 u-tiles' first k-halves early.
            for s in range(2):
                for u in range(NST):
                    nc.sync.dma_start(
                        wm_tiles[u][:, s * SL:(s + 1) * SL],
                        wmu[u][:, s * SL:(s + 1) * SL])
            # scalar ring: all x chunks (runs in parallel with the weight
            # stream on the sync ring — the slow early-DMA phase overlaps).
            for c in range(XC):
                s = slice(c * KC, (c + 1) * KC)
                nc.scalar.dma_start(x_sb[:, s, :], xT[:, s, :])

            def fetch_wm(u):
                wm = wp.tile([P, KT * P], BF16, tag="wm", bufs=10,
                             name=f"wm{u}")
                nc.sync.dma_start(wm[:], wmu[u])
                wm_tiles[u] = wm

            def fetch_ws(u):
                ws = wp.tile([P, KT * P], BF16, tag="ws", bufs=4,
                             name=f"ws{u}")
                nc.sync.dma_start(ws[:], wsg[u])
                ws_tiles[u] = ws

            for u in range(NST, NST + 4):
                fetch_wm(u)

            def fetch_ei(c):
                s = slice(c * 4, (c + 1) * 4)
                nc.scalar.dma_start(ei_sb[:, s, :], eiT[:, s, :])
                nc.vector.tensor_mul(z_sb[:, s, :], x_sb[:, s, :], ei_sb[:, s, :])

            def fetch_eo(c):
                s = slice(c * 4, (c + 1) * 4)
                nc.scalar.dma_start(eo_sb[:, s, :], eoT[:, s, :])

            # biases (tiny) on the gpsimd SWDGE queue.
            bmu_t = bp.tile([P, UT], FP32, tag="bmu")
            nc.gpsimd.dma_start(bmu_t[:], bmu[:])
            bsg_t = bp.tile([P, UT], FP32, tag="bsg")
            nc.gpsimd.dma_start(bsg_t[:], bsg[:])

            # ---- Phase 1a: k-chunk-major over u=0..3 while x streams in ----
            t_m = [None] * UT
            pm_st = [pp.tile([P, BL], FP32, tag=f"psm{u}", name=f"pm_st{u}")
                     for u in range(NST)]
            for c in range(XC):
                for u in range(NST):
                    wm = wm_tiles[u]
                    for k in range(c * KC, (c + 1) * KC):
                        nc.tensor.matmul(
                            pm_st[u][:], wm[:, k * P:(k + 1) * P], x_sb[:, k, :],
                            start=(k == 0), stop=(k == KT - 1),
                        )
            for u in range(NST):
                del wm_tiles[u]
                tm = mp.tile([P, BL], FP32, tag=f"tm{u}")
                nc.scalar.add(tm[:], pm_st[u][:], bmu_t[:, u:u + 1])
                t_m[u] = tm

            # ---- Phase 1b: mean terms u=4..15, u-major ----
            # eps_in/eps_out stream in the gaps of the weight stream.
            ei_at = {6: 0, 8: 1, 10: 2, 12: 3}
            eo_at = {13: 0, 14: 1, 15: 2}
            for u in range(NST, UT):
                if u + 4 < UT:
                    fetch_wm(u + 4)
                elif u + 4 == UT:
                    for uu in range(2):
                        fetch_ws(uu)
                if u in ei_at:
                    fetch_ei(ei_at[u])
                if u in eo_at:
                    fetch_eo(eo_at[u])
                wm = wm_tiles.pop(u)
                # u=4..6 borrow the (still idle) noise-phase PSUM banks so
                # their k-loops don't wait on the u=0..3 ACT drains.
                if u < NST + 3:
                    pm = ppn.tile([P, BL], FP32, tag="psn")
                else:
                    pm = pp.tile([P, BL], FP32, tag=f"psm{u % NST}")
                for k in range(KT):
                    nc.tensor.matmul(
                        pm[:], wm[:, k * P:(k + 1) * P], x_sb[:, k, :],
                        start=(k == 0), stop=(k == KT - 1),
                    )
                tm = mp.tile([P, BL], FP32, tag=f"tm{u}")
                nc.scalar.add(tm[:], pm[:], bmu_t[:, u:u + 1])
                t_m[u] = tm

            # ---- Phase 2: noise terms + combine ----
            for u in range(UT):
                un = u + 2
                if 2 <= un < UT:
                    fetch_ws(un)
                if u == 0:
                    fetch_eo(3)
                ws = ws_tiles.pop(u)
                # last two tiles: batch halves so the epilogue pipelines with
                # the final matmuls instead of serializing after them.
                halves = (0, BL // 2, BL) if u >= UT - 2 else (0, BL)
                for h in range(len(halves) - 1):
                    lo, hi = halves[h], halves[h + 1]
                    last_piece = (u == UT - 1 and h == len(halves) - 2)
                    pn = ppn.tile([P, hi - lo], FP32, tag="psn")
                    for k in range(KT):
                        nc.tensor.matmul(
                            pn[:], ws[:, k * P:(k + 1) * P], z_sb[:, k, lo:hi],
                            start=(k == 0), stop=(k == KT - 1),
                        )
                    t_n = tp.tile([P, hi - lo], FP32, tag="tn")
                    nc.scalar.add(t_n[:], pn[:], bsg_t[:, u:u + 1])
                    pr = tp.tile([P, hi - lo], FP32, tag="pr")
                    nc.vector.tensor_mul(pr[:], t_n[:], eo_sb[:, u, lo:hi])
                    o = op.tile([P, hi - lo], FP32, tag="o")
                    nc.vector.tensor_add(o[:], pr[:], t_m[u][:, lo:hi])
                    # outputs drain on the scalar ring; the final piece uses
                    # the (idle) sync ring so the last two out triggers run
                    # on different sequencers in parallel.
                    if last_piece:
                        nc.sync.dma_start(outT[u][:, lo:hi], o[:])
                    else:
                        nc.scalar.dma_start(outT[u][:, lo:hi], o[:])

    nc.compile()
    return nc


def _get_nc():
    global _cached
    if _cached is None:
        _cached = _build()
    return _cached


def kernel(x, weight_mu, weight_sigma, bias_mu, bias_sigma, eps_in, eps_out,
           _trace=False):
    nc = _get_nc()

    # Host-side layout prep (transposes + bf16 casts only; no layer math).
    def to_pkb(a):  # [B, D] -> per-core [P, KT, BL] (partition p holds k*128+p)
        a = np.ascontiguousarray(a.astype(_NBF))
        return [
            np.ascontiguousarray(
                a[c * BL:(c + 1) * BL].T.reshape(KT, P, BL).transpose(1, 0, 2))
            for c in range(N_CORES)
        ]

    xs = to_pkb(x)
    eis = to_pkb(eps_in)
    eos = to_pkb(eps_out)  # same transform, u in place of k

    def w_blocks(w):  # [D, U] -> [UT, P(d within block), KT*P] bf16
        wb = w.astype(_NBF).reshape(KT, P, UT, P).transpose(2, 1, 0, 3)
        return np.ascontiguousarray(wb.reshape(UT, P, KT * P))

    wmu_h = w_blocks(weight_mu)
    wsg_h = w_blocks(weight_sigma)
    bmu_h = np.ascontiguousarray(bias_mu.astype(np.float32).reshape(UT, P).T)
    bsg_h = np.ascontiguousarray(bias_sigma.astype(np.float32).reshape(UT, P).T)

    in_maps = [
        {
            "xT": xs[c],
            "eiT": eis[c],
            "eoT": eos[c],
            "wmu": wmu_h,
            "wsg": wsg_h,
            "bmu": bmu_h,
            "bsg": bsg_h,
        }
        for c in range(N_CORES)
    ]

    res = run_bass_kernel_spmd(nc, in_maps, core_ids=list(range(N_CORES)),
                               trace=_trace)
    kernel.last_result = res

    out = np.empty((B, U), dtype=np.float32)
    for c in range(N_CORES):
        oc = res.results[c]["outT"]  # [UT, P, BL]
        out[c * BL:(c + 1) * BL] = oc.transpose(2, 0, 1).reshape(BL, U)
    return out


# revision 21
# speedup vs baseline: 1.0746x; 1.0103x over previous
"""NoisyNet dense layer (training mode) on 8 TRN2 NeuronCores.

out[b,u] = x @ W_mu + eps_out * ((x*eps_in) @ W_sigma) + bias_mu + bias_sigma*eps_out

Sharding: data-parallel over batch (4096 -> 512 rows/core), weights/biases
replicated. On-device math runs in a transposed layout ([D,B]/[U,B]) so the
contraction dim D lands on SBUF partitions; the host does the (free)
transposes, bf16 casts and the final gather.

Schedule (v2): the two HWDGE rings are dedicated — sync carries the weight
stream, scalar carries x first, then eps_in/eps_out, then half the output
tiles (the other half drain on sync after the weights finish). Startup runs
k-chunk-major over the first 4 u-tiles so the PE consumes each arriving x
chunk against 4 weight tiles (8 matmuls per 256KB chunk) instead of
starving on a single u-tile k-loop. Warm-up matmuls on a vector-memset
tile cover the DMA latency + PE p-state ramp. The last two u-tiles run in
256-wide halves so the epilogue pipelines with the final matmuls.
"""

import numpy as np
import ml_dtypes

import concourse.bacc as bacc
import concourse.mybir as mybir
import concourse.tile as tile
from concourse.bass_utils import run_bass_kernel_spmd

N_CORES = 8
B, D, U = 4096, 2048, 2048
BL = B // N_CORES          # 512 batch rows per core
P = 128                    # partitions
KT = D // P                # 16 contraction tiles
UT = U // P                # 16 output tiles
XC = 8                     # x DMA chunks (2 k-tiles each)
WSL = 4                    # wm0-3 arrive in 4 k-slices each
NST = 4                    # u-tiles processed k-chunk-major at startup
BF16 = mybir.dt.bfloat16
FP32 = mybir.dt.float32

_NBF = ml_dtypes.bfloat16

_cached = None


def _build():
    nc = bacc.Bacc("TRN2", target_bir_lowering=False, debug=False)

    # activations laid out [P, KT, BL]: partition p holds d = k*128+p
    xT = nc.declare_dram_parameter("xT", [P, KT, BL], BF16, isOutput=False)
    eiT = nc.declare_dram_parameter("eiT", [P, KT, BL], BF16, isOutput=False)
    eoT = nc.declare_dram_parameter("eoT", [P, UT, BL], BF16, isOutput=False)
    wmu = nc.declare_dram_parameter("wmu", [UT, P, KT * P], BF16, isOutput=False)
    wsg = nc.declare_dram_parameter("wsg", [UT, P, KT * P], BF16, isOutput=False)
    bmu = nc.declare_dram_parameter("bmu", [P, UT], FP32, isOutput=False)
    bsg = nc.declare_dram_parameter("bsg", [P, UT], FP32, isOutput=False)
    outT = nc.declare_dram_parameter("outT", [UT, P, BL], FP32, isOutput=True)

    with tile.TileContext(nc) as tc:
        with (
            tc.tile_pool(name="acts", bufs=1) as acts,
            tc.tile_pool(name="w", bufs=6) as wp,
            tc.tile_pool(name="bias", bufs=1) as bp,
            tc.tile_pool(name="psum", bufs=1, space="PSUM") as pp,
            tc.tile_pool(name="psumn", bufs=3, space="PSUM") as ppn,
            tc.tile_pool(name="mean", bufs=1) as mp,
            tc.tile_pool(name="tmp", bufs=2) as tp,
            tc.tile_pool(name="out", bufs=3) as op,
        ):
            # ---- DMA issue (program order == ring FIFO order per engine) ----
            # sync ring: x chunks interleaved with wm0-3 k-slices, in the
            # exact order phase 1a consumes them. The sync ring's first
            # trigger fires ~1.3us before the scalar ring's (the scalar
            # engine runs ACT_TABLE_LOAD first), so the critical stream
            # lives here. Then wm4..7; wm8..15 + ws0..15 from the loops.
            x_sb = acts.tile([P, KT, BL], BF16, tag="x")
            ei_sb = acts.tile([P, KT, BL], BF16, tag="ei")
            z_sb = acts.tile([P, KT, BL], BF16, tag="z")
            eo_sb = acts.tile([P, UT, BL], BF16, tag="eo")

            wm_tiles = {}
            ws_tiles = {}
            for u in range(NST):
                wm_tiles[u] = wp.tile([P, KT * P], BF16, tag="wm", bufs=9,
                                      name=f"wm_st{u}")
            KC = KT // XC             # 2 k-tiles per x chunk
            SL = KT * P // 2          # 1024 cols per wm slice (8 k-tiles)

            # sync ring: wm0-3 in 2 k-slices each, slice-major so the
            # startup phase gets all 4 u-tiles' first k-halves early.
            for s in range(2):
                for u in range(NST):
                    nc.sync.dma_start(
                        wm_tiles[u][:, s * SL:(s + 1) * SL],
                        wmu[u][:, s * SL:(s + 1) * SL])
            # x_c0 on the gpsimd (SWDGE) ring, remaining chunks on the scalar
            # ring: all three rings pay their startup latency in parallel.
            nc.gpsimd.dma_start(x_sb[:, 0:KC, :], xT[:, 0:KC, :])
            for c in range(1, XC):
                s = slice(c * KC, (c + 1) * KC)
                nc.scalar.dma_start(x_sb[:, s, :], xT[:, s, :])

            def fetch_wm(u):
                wm = wp.tile([P, KT * P], BF16, tag="wm", bufs=9,
                             name=f"wm{u}")
                nc.sync.dma_start(wm[:], wmu[u])
                wm_tiles[u] = wm

            def fetch_ws(u):
                ws = wp.tile([P, KT * P], BF16, tag="ws", bufs=4,
                             name=f"ws{u}")
                nc.sync.dma_start(ws[:], wsg[u])
                ws_tiles[u] = ws

            for u in range(NST, NST + 4):
                fetch_wm(u)

            def fetch_ei(c):
                s = slice(c * 4, (c + 1) * 4)
                nc.scalar.dma_start(ei_sb[:, s, :], eiT[:, s, :])
                nc.vector.tensor_mul(z_sb[:, s, :], x_sb[:, s, :], ei_sb[:, s, :])

            def fetch_eo(c):
                s = slice(c * 4, (c + 1) * 4)
                nc.scalar.dma_start(eo_sb[:, s, :], eoT[:, s, :])

            # biases (tiny) on the gpsimd SWDGE queue.
            bmu_t = bp.tile([P, UT], FP32, tag="bmu")
            nc.gpsimd.dma_start(bmu_t[:], bmu[:])
            bsg_t = bp.tile([P, UT], FP32, tag="bsg")
            nc.gpsimd.dma_start(bsg_t[:], bsg[:])

            # ---- Phase 1a: k-chunk-major over u=0..3 while x streams in ----
            t_m = [None] * UT
            pm_st = [pp.tile([P, BL], FP32, tag=f"psm{u}", name=f"pm_st{u}")
                     for u in range(NST)]
            for c in range(XC):
                for u in range(NST):
                    wm = wm_tiles[u]
                    for k in range(c * KC, (c + 1) * KC):
                        nc.tensor.matmul(
                            pm_st[u][:], wm[:, k * P:(k + 1) * P], x_sb[:, k, :],
                            start=(k == 0), stop=(k == KT - 1),
                        )
            # All drains run on the vector engine: with zero InstActivation
            # in the program the framework skips ACT_TABLE_LOAD, so the
            # scalar ring's first DMA trigger fires ~1.3us earlier.
            for u in range(NST):
                del wm_tiles[u]
                tm = mp.tile([P, BL], FP32, tag=f"tm{u}")
                nc.vector.tensor_scalar_add(tm[:], pm_st[u][:],
                                            bmu_t[:, u:u + 1])
                t_m[u] = tm

            # ---- Phase 1b: mean terms u=4..15, u-major ----
            # eps_in/eps_out stream in the gaps of the weight stream.
            ei_at = {6: 0, 8: 1, 10: 2, 12: 3}
            eo_at = {13: 0, 14: 1, 15: 2}
            for u in range(NST, UT):
                if u + 4 < UT:
                    fetch_wm(u + 4)
                elif u + 4 == UT:
                    for uu in range(2):
                        fetch_ws(uu)
                if u in ei_at:
                    fetch_ei(ei_at[u])
                if u in eo_at:
                    fetch_eo(eo_at[u])
                wm = wm_tiles.pop(u)
                # u=4..6 borrow the (still idle) noise-phase PSUM banks so
                # their k-loops don't wait on the u=0..3 ACT drains.
                if u < NST + 3:
                    pm = ppn.tile([P, BL], FP32, tag="psn")
                else:
                    pm = pp.tile([P, BL], FP32, tag=f"psm{u % NST}")
                for k in range(KT):
                    nc.tensor.matmul(
                        pm[:], wm[:, k * P:(k + 1) * P], x_sb[:, k, :],
                        start=(k == 0), stop=(k == KT - 1),
                    )
                tm = mp.tile([P, BL], FP32, tag=f"tm{u}")
                nc.vector.tensor_scalar_add(tm[:], pm[:], bmu_t[:, u:u + 1])
                t_m[u] = tm

            # ---- Phase 2: noise terms + combine ----
            for u in range(UT):
                un = u + 2
                if 2 <= un < UT:
                    fetch_ws(un)
                if u == 0:
                    fetch_eo(3)
                ws = ws_tiles.pop(u)
                # Precompute tmb = t_m + bias_sigma*eps_out off the critical
                # path: the per-piece drain is then 2 ops (mul, add) instead
                # of 3, shortening the tail chain after the last matmul.
                bse = tp.tile([P, BL], FP32, tag="bse")
                nc.vector.tensor_scalar_mul(bse[:], eo_sb[:, u, :],
                                            bsg_t[:, u:u + 1])
                tmb = mp.tile([P, BL], FP32, tag=f"tmb{u}")
                nc.vector.tensor_add(tmb[:], t_m[u][:], bse[:])
                # last two tiles: batch halves so the epilogue pipelines with
                # the final matmuls instead of serializing after them.
                halves = (0, BL // 2, BL) if u >= UT - 2 else (0, BL)
                for h in range(len(halves) - 1):
                    lo, hi = halves[h], halves[h + 1]
                    last_piece = (u == UT - 1 and h == len(halves) - 2)
                    pn = ppn.tile([P, hi - lo], FP32, tag="psn")
                    for k in range(KT):
                        nc.tensor.matmul(
                            pn[:], ws[:, k * P:(k + 1) * P], z_sb[:, k, lo:hi],
                            start=(k == 0), stop=(k == KT - 1),
                        )
                    pr = tp.tile([P, hi - lo], FP32, tag="pr")
                    nc.vector.tensor_mul(pr[:], pn[:], eo_sb[:, u, lo:hi])
                    o = op.tile([P, hi - lo], FP32, tag="o")
                    nc.vector.tensor_add(o[:], pr[:], tmb[:, lo:hi])
                    # outputs drain on the scalar ring; the final piece uses
                    # the (idle) sync ring so the last two out triggers run
                    # on different sequencers in parallel.
                    if last_piece:
                        nc.sync.dma_start(outT[u][:, lo:hi], o[:])
                    else:
                        nc.scalar.dma_start(outT[u][:, lo:hi], o[:])

    nc.compile()
    return nc


def _get_nc():
    global _cached
    if _cached is None:
        _cached = _build()
    return _cached


def kernel(x, weight_mu, weight_sigma, bias_mu, bias_sigma, eps_in, eps_out,
           _trace=False):
    nc = _get_nc()

    # Host-side layout prep (transposes + bf16 casts only; no layer math).
    def to_pkb(a):  # [B, D] -> per-core [P, KT, BL] (partition p holds k*128+p)
        a = np.ascontiguousarray(a.astype(_NBF))
        return [
            np.ascontiguousarray(
                a[c * BL:(c + 1) * BL].T.reshape(KT, P, BL).transpose(1, 0, 2))
            for c in range(N_CORES)
        ]

    xs = to_pkb(x)
    eis = to_pkb(eps_in)
    eos = to_pkb(eps_out)  # same transform, u in place of k

    def w_blocks(w):  # [D, U] -> [UT, P(d within block), KT*P] bf16
        wb = w.astype(_NBF).reshape(KT, P, UT, P).transpose(2, 1, 0, 3)
        return np.ascontiguousarray(wb.reshape(UT, P, KT * P))

    wmu_h = w_blocks(weight_mu)
    wsg_h = w_blocks(weight_sigma)
    bmu_h = np.ascontiguousarray(bias_mu.astype(np.float32).reshape(UT, P).T)
    bsg_h = np.ascontiguousarray(bias_sigma.astype(np.float32).reshape(UT, P).T)

    in_maps = [
        {
            "xT": xs[c],
            "eiT": eis[c],
            "eoT": eos[c],
            "wmu": wmu_h,
            "wsg": wsg_h,
            "bmu": bmu_h,
            "bsg": bsg_h,
        }
        for c in range(N_CORES)
    ]

    res = run_bass_kernel_spmd(nc, in_maps, core_ids=list(range(N_CORES)),
                               trace=_trace)
    kernel.last_result = res

    out = np.empty((B, U), dtype=np.float32)
    for c in range(N_CORES):
        oc = res.results[c]["outT"]  # [UT, P, BL]
        out[c * BL:(c + 1) * BL] = oc.transpose(2, 0, 1).reshape(BL, U)
    return out
